# revision 17
# baseline (speedup 1.0000x reference)
"""Trainium2 Bass kernel for LowRankSparseHyperedgeGen (moe_routing).

Contract: kernel(**inputs) takes FULL unsharded inputs, returns FULL output
(edge_idx [B,N,kE] i32, edge_w [B,N,kE] f32, E).

Sharding: batch B=8 across the 8 NeuronCores (1 sample/core); all weights
replicated. Two SPMD launches:
  K1: per-sample context (mean/max over nodes), dynamic low-rank factor
      V_dyn, and hyperedge scores [E] (all f32 on device).
  host: top-512 selection per sample (tiny: argsort of 2048) + row gathers.
  K2: X head-projection (transposed via PE), proto.T reconstruction, logits
      matmul, softmax, and materialization of edge_w / edge_idx.
"""

import numpy as np
from contextlib import ExitStack

import concourse.bass as bass
import concourse.bacc as bacc
import concourse.mybir as mybir
import concourse.tile as tile
from concourse.bass_utils import run_bass_kernel_spmd
from concourse.masks import make_identity

F32 = mybir.dt.float32
F32R = mybir.dt.float32r
I32 = mybir.dt.int32

B, N, D, E, R = 8, 4096, 256, 2048, 16
H = 4
KE = 512            # kE = E * 0.25
DH = D // H
SCALE = 1.0 / (np.sqrt(DH) * H)   # 1/32 logit scale
NT = N // 128       # 32 node tiles
NCORES = 8

# float32r streams fp32 at full PE rate (1 cyc/row when moving dim >= 256);
# inputs must be produced as float32r (compute-op outputs convert). Only K2's
# big matmuls use it; K1 scoring stays plain fp32 for exact top-k ranking.
USE_F32R_K2 = True
MMDT = F32R if USE_F32R_K2 else F32


# ---------------------------------------------------------------- K1 -------
def build_k1():
    nc = bacc.Bacc()
    X = nc.dram_tensor("X", [N, D], F32, kind="ExternalInput")
    WcT = nc.dram_tensor("WcT", [2 * D, R * D], F32, kind="ExternalInput")
    WsT = nc.dram_tensor("WsT", [2 * D, D], F32, kind="ExternalInput")
    UT = nc.dram_tensor("UT", [R, E], F32, kind="ExternalInput")
    pbT = nc.dram_tensor("pbT", [D, E], F32, kind="ExternalInput")
    Vbc = nc.dram_tensor("Vbc", [R, D], F32, kind="ExternalInput")
    bs = nc.dram_tensor("bs", [D], F32, kind="ExternalInput")
    scores = nc.dram_tensor("scores", [E], F32, kind="ExternalOutput")
    vdyn = nc.dram_tensor("vdyn", [R, D], F32, kind="ExternalOutput")

    with tile.TileContext(nc) as tc, ExitStack() as ctx:
        singles = ctx.enter_context(tc.tile_pool(name="singles", bufs=1))
        xpool = ctx.enter_context(tc.tile_pool(name="x", bufs=4))
        wpool = ctx.enter_context(tc.tile_pool(name="w", bufs=4))
        small = ctx.enter_context(tc.tile_pool(name="small", bufs=2))
        pred = ctx.enter_context(tc.tile_pool(name="pred", bufs=2, space="PSUM"))
        pmm = ctx.enter_context(tc.tile_pool(name="pmm", bufs=2, space="PSUM"))

        id128 = singles.tile([128, 128], F32)
        make_identity(nc, id128)

        # ---- streaming mean/max over nodes ----
        acc_sum = singles.tile([128, D], F32)
        acc_max = singles.tile([128, D], F32)
        for i in range(NT):
            xt = xpool.tile([128, D], F32)
            nc.sync.dma_start(out=xt, in_=X[128 * i:128 * (i + 1), :])
            if i == 0:
                nc.vector.tensor_copy(acc_sum, xt)
                nc.vector.tensor_copy(acc_max, xt)
            else:
                nc.vector.tensor_add(acc_sum, acc_sum, xt)
                nc.vector.tensor_max(acc_max, acc_max, xt)

        # ctx vector in partition layout: 4 chunks of [128,1]
        # j=0,1: mean chunks; j=2,3: max chunks
        ctx_k = [singles.tile([128, 1], F32, tag=f"ctx{j}", name=f"ctx{j}") for j in range(4)]
        for t in range(2):
            pst = pred.tile([128, 128], F32, tag="ptr")
            nc.tensor.transpose(pst, acc_sum[:, 128 * t:128 * (t + 1)], id128)
            ssum = small.tile([128, 1], F32, tag="ssum")
            nc.vector.tensor_reduce(ssum, pst, axis=mybir.AxisListType.X,
                                    op=mybir.AluOpType.add)
            nc.vector.tensor_scalar_mul(ctx_k[t], ssum, 1.0 / N)
            pst2 = pred.tile([128, 128], F32, tag="ptr")
            nc.tensor.transpose(pst2, acc_max[:, 128 * t:128 * (t + 1)], id128)
            nc.vector.tensor_reduce(ctx_k[2 + t], pst2, axis=mybir.AxisListType.X,
                                    op=mybir.AluOpType.max)

        # ---- wdyn = ctx @ Wc.T  -> [1, R*D] ----
        wdyn_sb = singles.tile([1, R * D], F32)
        for n in range(8):
            ps = pmm.tile([1, 512], F32, tag="pmm512")
            for j in range(4):
                wt = wpool.tile([128, 512], F32)
                nc.sync.dma_start(
                    out=wt, in_=WcT[128 * j:128 * (j + 1), 512 * n:512 * (n + 1)])
                nc.tensor.matmul(ps, ctx_k[j], wt, start=(j == 0), stop=(j == 3))
            nc.scalar.copy(wdyn_sb[:, 512 * n:512 * (n + 1)], ps)

        # ---- vdyn_nat [R, D] = V + bc.reshape + wdyn.reshape ----
        vbc_sb = singles.tile([R, D], F32)
        nc.sync.dma_start(out=vbc_sb, in_=Vbc[:, :])
        vnat = singles.tile([R, D], F32)
        for r in range(R):
            nc.sync.dma_start(out=vnat[r:r + 1, :],
                              in_=wdyn_sb[0:1, D * r:D * (r + 1)])
        nc.vector.tensor_add(vnat, vnat, vbc_sb)
        nc.sync.dma_start(out=vdyn[:, :], in_=vnat)

        # vdynT chunks [128, R] via PE transpose of vnat
        id16 = id128[:R, :R]
        vdynT = []
        for t in range(2):
            pvt = pred.tile([128, R], F32, tag="ptr")
            nc.tensor.transpose(pvt, vnat[:, 128 * t:128 * (t + 1)], id16)
            vt = singles.tile([128, R], F32, tag=f"vdynT{t}", name=f"vdynT{t}")
            nc.scalar.copy(vt, pvt)
            vdynT.append(vt)

        # ---- sc = ctx @ Ws.T + bs -> [D] in partition chunks [128,1] x2 ----
        sc_sb = []
        for t in range(2):
            pssc = pmm.tile([128, 1], F32, tag="pmmtiny")
            for j in range(4):
                wst = wpool.tile([128, D], F32, tag="wst")
                nc.sync.dma_start(out=wst, in_=WsT[128 * j:128 * (j + 1), :])
                nc.tensor.matmul(pssc, wst[:, 128 * t:128 * (t + 1)], ctx_k[j],
                                 start=(j == 0), stop=(j == 3))
            bst = small.tile([128, 1], F32)
            nc.sync.dma_start(out=bst, in_=bs[128 * t:128 * (t + 1)])
            sct = singles.tile([128, 1], F32, tag=f"sc{t}", name=f"sct{t}")
            nc.vector.tensor_add(sct, pssc, bst)
            sc_sb.append(sct)

        # ---- z = V_dyn @ sc -> [R,1] ----
        psz = pmm.tile([R, 1], F32, tag="pmmtiny")
        for t in range(2):
            nc.tensor.matmul(psz, vdynT[t], sc_sb[t], start=(t == 0), stop=(t == 1))
        z_sb = singles.tile([R, 1], F32)
        nc.scalar.copy(z_sb, psz)

        # ---- scores = U @ z + proto_bias @ sc -> [1, E] ----
        ut_sb = singles.tile([R, E], F32)
        nc.sync.dma_start(out=ut_sb, in_=UT[:, :])
        sc_out = singles.tile([1, E], F32)
        for c in range(E // 512):
            pss = pmm.tile([1, 512], F32, tag="pmm512")
            nc.tensor.matmul(pss, z_sb, ut_sb[:, 512 * c:512 * (c + 1)],
                             start=True, stop=False)
            for t in range(2):
                pbt = wpool.tile([128, 512], F32, tag="pbt")
                nc.sync.dma_start(
                    out=pbt, in_=pbT[128 * t:128 * (t + 1), 512 * c:512 * (c + 1)])
                nc.tensor.matmul(pss, sc_sb[t], pbt, start=False, stop=(t == 1))
            nc.scalar.copy(sc_out[:, 512 * c:512 * (c + 1)], pss)
        nc.sync.dma_start(out=scores[:], in_=sc_out)

    nc.compile()
    return nc


# ---------------------------------------------------------------- K2 -------
def build_k2():
    nc = bacc.Bacc()
    X = nc.dram_tensor("X", [N, D], F32, kind="ExternalInput")
    WpT = nc.dram_tensor("WpT", [D, D], F32, kind="ExternalInput")
    bp = nc.dram_tensor("bp", [D], F32, kind="ExternalInput")
    vdyn_s = nc.dram_tensor("vdyn_s", [R, D], F32, kind="ExternalInput")
    UselT = nc.dram_tensor("UselT", [R, KE], F32, kind="ExternalInput")
    bselT = nc.dram_tensor("bselT", [D, KE], F32, kind="ExternalInput")
    idxbc = nc.dram_tensor("idxbc", [128, KE], I32, kind="ExternalInput")
    edge_w = nc.dram_tensor("edge_w", [N, KE], F32, kind="ExternalOutput")
    edge_idx = nc.dram_tensor("edge_idx", [N, KE], I32, kind="ExternalOutput")

    with tile.TileContext(nc) as tc, ExitStack() as ctx:
        singles = ctx.enter_context(tc.tile_pool(name="singles", bufs=1))
        xpool = ctx.enter_context(tc.tile_pool(name="x", bufs=4))
        xtg_pool = ctx.enter_context(tc.tile_pool(name="xtg", bufs=4))
        xwg_pool = ctx.enter_context(tc.tile_pool(name="xwg", bufs=4))
        ew_pool = ctx.enter_context(tc.tile_pool(name="ew", bufs=4))
        small = ctx.enter_context(tc.tile_pool(name="small", bufs=8))
        ptr = ctx.enter_context(tc.tile_pool(name="ptr", bufs=2, space="PSUM"))
        pshared = ctx.enter_context(tc.tile_pool(name="psh", bufs=2, space="PSUM"))
        plog = ctx.enter_context(tc.tile_pool(name="plog", bufs=3, space="PSUM"))

        id128 = singles.tile([128, 128], F32)
        make_identity(nc, id128)

        wpt = []
        for t in range(2):
            w0 = singles.tile([128, D], F32, tag=f"wpt0_{t}", name=f"wpt0_{t}")
            nc.sync.dma_start(out=w0, in_=WpT[128 * t:128 * (t + 1), :])
            w = singles.tile([128, D], MMDT, tag=f"wpt{t}", name=f"wpt{t}")
            nc.vector.tensor_copy(w, w0)
            wpt.append(w)

        bp_sb = []
        for t in range(2):
            b = singles.tile([128, 1], F32, tag=f"bp{t}", name=f"bpt{t}")
            nc.sync.dma_start(out=b, in_=bp[128 * t:128 * (t + 1)])
            bp_sb.append(b)

        vd0 = singles.tile([R, D], F32)
        nc.sync.dma_start(out=vd0, in_=vdyn_s[:, :])
        vd = singles.tile([R, D], MMDT)
        nc.vector.tensor_copy(vd, vd0)
        us0 = singles.tile([R, KE], F32)
        nc.sync.dma_start(out=us0, in_=UselT[:, :])
        us = singles.tile([R, KE], MMDT)
        nc.vector.tensor_copy(us, us0)

        idx_sb = singles.tile([128, KE], I32)
        nc.sync.dma_start(out=idx_sb, in_=idxbc[:, :])

        # protoT [D, KE] scaled by 1/32 (host pre-scales vdyn_s and bselT)
        protoT = []
        for t in range(2):
            bsel = singles.tile([128, KE], F32, tag=f"bsel{t}", name=f"bsel{t}")
            nc.sync.dma_start(out=bsel, in_=bselT[128 * t:128 * (t + 1), :])
            pp = pshared.tile([128, KE], F32, tag="pxw")
            nc.tensor.matmul(pp, vd[:, 128 * t:128 * (t + 1)], us[:, :],
                             start=True, stop=True)
            pt = singles.tile([128, KE], MMDT, tag=f"protoT{t}", name=f"protoT{t}")
            nc.vector.tensor_add(pt, pp, bsel)
            protoT.append(pt)

        for g in range(N // 512):
            # transpose X group -> xtg[t] = X.T [din 128t.., 512 nodes]
            xtg = [xtg_pool.tile([128, 512], MMDT, tag=f"xtg{t}", name=f"xtg{t}") for t in range(2)]
            for j in range(4):
                xt = xpool.tile([128, D], F32)
                nc.sync.dma_start(
                    out=xt, in_=X[512 * g + 128 * j:512 * g + 128 * (j + 1), :])
                for t in range(2):
                    pst = ptr.tile([128, 128], F32)
                    nc.tensor.transpose(pst, xt[:, 128 * t:128 * (t + 1)], id128)
                    nc.scalar.copy(xtg[t][:, 128 * j:128 * (j + 1)], pst)

            # XWp.T for this group: [dout 128t.., 512 nodes]
            xwg = [xwg_pool.tile([128, 512], MMDT, tag=f"xwg{t}", name=f"xwg{t}") for t in range(2)]
            for t in range(2):
                pxw = pshared.tile([128, KE], F32, tag="pxw")
                for j in range(2):
                    nc.tensor.matmul(pxw, wpt[j][:, 128 * t:128 * (t + 1)],
                                     xtg[j], start=(j == 0), stop=(j == 1))
                nc.vector.tensor_scalar_add(xwg[t], pxw, bp_sb[t])

            # logits + softmax per 128-node tile
            for j in range(4):
                pl = plog.tile([128, KE], F32)
                for t in range(2):
                    nc.tensor.matmul(pl, xwg[t][:, 128 * j:128 * (j + 1)],
                                     protoT[t], start=(t == 0), stop=(t == 1))
                negmax = small.tile([128, 1], F32, tag="negmax")
                nc.vector.tensor_reduce(negmax, pl, axis=mybir.AxisListType.X,
                                        op=mybir.AluOpType.max, negate=True)
                ew = ew_pool.tile([128, KE], F32)
                sumexp = small.tile([128, 1], F32, tag="sumexp")
                nc.scalar.activation(ew, pl, mybir.ActivationFunctionType.Exp,
                                     bias=negmax, scale=1.0, accum_out=sumexp)
                rinv = small.tile([128, 1], F32, tag="rinv")
                nc.vector.reciprocal(rinv, sumexp)
                nc.vector.tensor_scalar_mul(ew, ew, rinv)
                row0 = 512 * g + 128 * j
                nc.sync.dma_start(out=edge_w[row0:row0 + 128, :], in_=ew)
                nc.sync.dma_start(out=edge_idx[row0:row0 + 128, :], in_=idx_sb)

    nc.compile()
    return nc


# ------------------------------------------------------------- host --------
def _ensure_profile_hook():
    """Register the axon NTFF profiling hook if antenv.axon_hooks is absent
    (the boot shim only registers it when the module exists)."""
    import sys
    import types
    try:
        from antenv.axon_hooks import get_axon_ntff_profile_hook  # noqa: F401
        return
    except ImportError:
        pass
    import antenv
    mod = types.ModuleType("antenv.axon_hooks")
    mod._hook = None
    mod.set_axon_ntff_profile_hook = lambda h: setattr(mod, "_hook", h)
    mod.get_axon_ntff_profile_hook = lambda: mod._hook
    sys.modules["antenv.axon_hooks"] = mod
    antenv.axon_hooks = mod
    try:
        from trn_agent_boot.trn_boot import _ntff_profile_via_ctypes
        mod._hook = _ntff_profile_via_ctypes("/opt/axon/libaxon_pjrt.so")
    except Exception:
        mod._hook = None


_CACHE = {}


def _get_k1():
    if "k1" not in _CACHE:
        _CACHE["k1"] = build_k1()
    return _CACHE["k1"]


def _get_k2():
    if "k2" not in _CACHE:
        _CACHE["k2"] = build_k2()
    return _CACHE["k2"]


def kernel(X, U, V, proto_bias, Wc, bc, Ws, bs, Wp, bp, _trace=False):
    X = np.asarray(X, np.float32)
    U = np.asarray(U, np.float32)
    V = np.asarray(V, np.float32)
    proto_bias = np.asarray(proto_bias, np.float32)
    Wc = np.asarray(Wc, np.float32)
    bc = np.asarray(bc, np.float32)
    Ws = np.asarray(Ws, np.float32)
    bs = np.asarray(bs, np.float32)
    Wp = np.asarray(Wp, np.float32)
    bp = np.asarray(bp, np.float32)

    if _trace:
        _ensure_profile_hook()

    WcT = np.ascontiguousarray(Wc.T)
    WsT = np.ascontiguousarray(Ws.T)
    UT = np.ascontiguousarray(U.T)
    pbT = np.ascontiguousarray(proto_bias.T)
    Vbc = np.ascontiguousarray(V + bc.reshape(R, D))

    in_maps1 = []
    for b in range(B):
        in_maps1.append({
            "X": np.ascontiguousarray(X[b]),
            "WcT": WcT, "WsT": WsT, "UT": UT, "pbT": pbT,
            "Vbc": Vbc, "bs": bs,
        })
    res1 = run_bass_kernel_spmd(_get_k1(), in_maps1,
                                core_ids=list(range(NCORES)), trace=_trace)
    t1 = res1.exec_time_ns

    WpT = np.ascontiguousarray(Wp.T)
    in_maps2 = []
    topk = np.empty((B, KE), np.int32)
    for b in range(B):
        s = res1.results[b]["scores"]
        idx = np.argsort(-s, kind="stable")[:KE].astype(np.int32)
        topk[b] = idx
        vdyn_b = res1.results[b]["vdyn"]
        in_maps2.append({
            "X": np.ascontiguousarray(X[b]),
            "WpT": WpT,
            "bp": bp,
            "vdyn_s": np.ascontiguousarray(vdyn_b * SCALE),
            "UselT": np.ascontiguousarray(U[idx].T),
            "bselT": np.ascontiguousarray(proto_bias[idx].T * SCALE),
            "idxbc": np.ascontiguousarray(
                np.broadcast_to(idx[None, :], (128, KE))),
        })
    res2 = run_bass_kernel_spmd(_get_k2(), in_maps2,
                                core_ids=list(range(NCORES)), trace=_trace)
    t2 = res2.exec_time_ns

    edge_w = np.stack([res2.results[b]["edge_w"] for b in range(B)])
    edge_idx = np.stack([res2.results[b]["edge_idx"] for b in range(B)])

    kernel.last_exec_ns = ((t1 or 0) + (t2 or 0)) or None
    kernel.last_scores = np.stack([res1.results[b]["scores"] for b in range(B)])
    return edge_idx, edge_w, E


kernel.last_exec_ns = None


# revision 18
# speedup vs baseline: 1.1320x; 1.1320x over previous
"""Trainium2 Bass kernel for LowRankSparseHyperedgeGen (moe_routing).

Contract: kernel(**inputs) takes FULL unsharded inputs, returns FULL output
(edge_idx [B,N,kE] i32, edge_w [B,N,kE] f32, E).

Sharding: batch B=8 across the 8 NeuronCores (1 sample/core); all weights
replicated. Two SPMD launches:
  K1: per-sample context (mean/max over nodes), dynamic low-rank factor
      V_dyn, and hyperedge scores [E] (all plain fp32 on device so the
      top-k ranking is bit-stable vs the fp32 reference).
  host: top-512 selection per sample (tiny: argsort of 2048) + row gathers.
  K2: X head-projection (transposed via PE), proto.T reconstruction, logits
      matmul (float32r = full-rate fp32 streaming), softmax, and
      materialization of edge_w / edge_idx.

DMA layout notes: X rows are packed 4-consecutive-rows-per-partition
([128, 4, D] tiles) and outputs are written one 512-row group at a time
([128, 4, KE] -> [512, KE]), so every DMA descriptor covers 4-16KB of
contiguous DRAM. Loads and stores are split between the two HWDGE queues
(sync + scalar) since descriptor generation runs on the issuing engine.
"""

import numpy as np
from contextlib import ExitStack

import concourse.bass as bass
import concourse.bacc as bacc
import concourse.mybir as mybir
import concourse.tile as tile
from concourse.bass_utils import run_bass_kernel_spmd
from concourse.masks import make_identity

F32 = mybir.dt.float32
F32R = mybir.dt.float32r
I32 = mybir.dt.int32

B, N, D, E, R = 8, 4096, 256, 2048, 16
H = 4
KE = 512            # kE = E * 0.25
DH = D // H
SCALE = 1.0 / (np.sqrt(DH) * H)   # 1/32 logit scale
NCORES = 8

# float32r streams fp32 at full PE rate (1 cyc/row when moving dim >= 256);
# inputs must be produced as float32r (compute-op outputs convert). Only K2's
# big matmuls use it; K1 scoring stays plain fp32 for exact top-k ranking.
MMDT = F32R


# ---------------------------------------------------------------- K1 -------
def build_k1():
    nc = bacc.Bacc()
    X = nc.dram_tensor("X", [N, D], F32, kind="ExternalInput")
    WcT = nc.dram_tensor("WcT", [2 * D, R * D], F32, kind="ExternalInput")
    WsT = nc.dram_tensor("WsT", [2 * D, D], F32, kind="ExternalInput")
    UT = nc.dram_tensor("UT", [R, E], F32, kind="ExternalInput")
    pbT = nc.dram_tensor("pbT", [D, E], F32, kind="ExternalInput")
    Vbc = nc.dram_tensor("Vbc", [R, D], F32, kind="ExternalInput")
    bs = nc.dram_tensor("bs", [D], F32, kind="ExternalInput")
    scores = nc.dram_tensor("scores", [E], F32, kind="ExternalOutput")
    vdyn = nc.dram_tensor("vdyn", [R, D], F32, kind="ExternalOutput")

    with tile.TileContext(nc) as tc, ExitStack() as ctx:
        singles = ctx.enter_context(tc.tile_pool(name="singles", bufs=1))
        xpool = ctx.enter_context(tc.tile_pool(name="x", bufs=4))
        small = ctx.enter_context(tc.tile_pool(name="small", bufs=2))
        pred = ctx.enter_context(tc.tile_pool(name="pred", bufs=2, space="PSUM"))
        pmm = ctx.enter_context(tc.tile_pool(name="pmm", bufs=2, space="PSUM"))

        id128 = singles.tile([128, 128], F32)
        make_identity(nc, id128)

        # ---- weights resident in SBUF (sync queue; 8-16KB descriptors) ----
        wc_sb = []
        for j in range(4):
            w = singles.tile([128, R * D], F32, tag=f"wc{j}", name=f"wc{j}")
            nc.sync.dma_start(out=w, in_=WcT[128 * j:128 * (j + 1), :])
            wc_sb.append(w)
        pb_sb = []
        for t in range(2):
            w = singles.tile([128, E], F32, tag=f"pb{t}", name=f"pb{t}")
            nc.sync.dma_start(out=w, in_=pbT[128 * t:128 * (t + 1), :])
            pb_sb.append(w)
        ws_sb = []
        for j in range(4):
            w = singles.tile([128, D], F32, tag=f"ws{j}", name=f"ws{j}")
            nc.sync.dma_start(out=w, in_=WsT[128 * j:128 * (j + 1), :])
            ws_sb.append(w)
        ut_sb = singles.tile([R, E], F32)
        nc.sync.dma_start(out=ut_sb, in_=UT[:, :])
        vbc_sb = singles.tile([R, D], F32)
        nc.sync.dma_start(out=vbc_sb, in_=Vbc[:, :])

        # ---- streaming mean/max over nodes (X on the scalar queue) ----
        # [128, 4, D] tiles: partition p holds rows 512k+4p .. 512k+4p+3.
        acc_sum = singles.tile([128, 4, D], F32)
        acc_max = singles.tile([128, 4, D], F32)
        for k in range(N // 512):
            xt = xpool.tile([128, 4, D], F32)
            nc.scalar.dma_start(out=xt, in_=X[512 * k:512 * (k + 1), :])
            if k == 0:
                nc.vector.tensor_copy(acc_sum, xt)
                nc.vector.tensor_copy(acc_max, xt)
            else:
                nc.vector.tensor_add(acc_sum, acc_sum, xt)
                nc.vector.tensor_max(acc_max, acc_max, xt)
        # fold the 4-row sub-axis
        red_sum = singles.tile([128, D], F32)
        red_max = singles.tile([128, D], F32)
        nc.vector.tensor_add(red_sum, acc_sum[:, 0, :], acc_sum[:, 1, :])
        nc.vector.tensor_add(red_sum, red_sum, acc_sum[:, 2, :])
        nc.vector.tensor_add(red_sum, red_sum, acc_sum[:, 3, :])
        nc.vector.tensor_max(red_max, acc_max[:, 0, :], acc_max[:, 1, :])
        nc.vector.tensor_max(red_max, red_max, acc_max[:, 2, :])
        nc.vector.tensor_max(red_max, red_max, acc_max[:, 3, :])

        # ctx vector in partition layout: 4 chunks of [128,1]
        # j=0,1: mean chunks; j=2,3: max chunks
        ctx_k = [singles.tile([128, 1], F32, tag=f"ctx{j}", name=f"ctx{j}")
                 for j in range(4)]
        for t in range(2):
            pst = pred.tile([128, 128], F32, tag="ptr")
            nc.tensor.transpose(pst, red_sum[:, 128 * t:128 * (t + 1)], id128)
            ssum = small.tile([128, 1], F32, tag="ssum")
            nc.vector.tensor_reduce(ssum, pst, axis=mybir.AxisListType.X,
                                    op=mybir.AluOpType.add)
            nc.vector.tensor_scalar_mul(ctx_k[t], ssum, 1.0 / N)
            pst2 = pred.tile([128, 128], F32, tag="ptr")
            nc.tensor.transpose(pst2, red_max[:, 128 * t:128 * (t + 1)], id128)
            nc.vector.tensor_reduce(ctx_k[2 + t], pst2, axis=mybir.AxisListType.X,
                                    op=mybir.AluOpType.max)

        # ---- wdyn = ctx @ Wc.T  -> [1, R*D] (plain fp32 on PE) ----
        wdyn_sb = singles.tile([1, R * D], F32)
        for n in range(8):
            ps = pmm.tile([1, 512], F32, tag="pmm512")
            for j in range(4):
                nc.tensor.matmul(ps, ctx_k[j], wc_sb[j][:, 512 * n:512 * (n + 1)],
                                 start=(j == 0), stop=(j == 3))
            nc.scalar.copy(wdyn_sb[:, 512 * n:512 * (n + 1)], ps)

        # ---- vdyn_nat [R, D] = V + bc.reshape + wdyn.reshape ----
        vnat = singles.tile([R, D], F32)
        for r in range(R):
            nc.sync.dma_start(out=vnat[r:r + 1, :],
                              in_=wdyn_sb[0:1, D * r:D * (r + 1)])
        nc.vector.tensor_add(vnat, vnat, vbc_sb)
        nc.sync.dma_start(out=vdyn[:, :], in_=vnat)

        # vdynT chunks [128, R] via PE transpose of vnat
        id16 = id128[:R, :R]
        vdynT = []
        for t in range(2):
            pvt = pred.tile([128, R], F32, tag="ptr")
            nc.tensor.transpose(pvt, vnat[:, 128 * t:128 * (t + 1)], id16)
            vt = singles.tile([128, R], F32, tag=f"vdynT{t}", name=f"vdynT{t}")
            nc.scalar.copy(vt, pvt)
            vdynT.append(vt)

        # ---- sc = ctx @ Ws.T + bs -> [D] in partition chunks [128,1] x2 ----
        sc_sb = []
        for t in range(2):
            pssc = pmm.tile([128, 1], F32, tag="pmmtiny")
            for j in range(4):
                nc.tensor.matmul(pssc, ws_sb[j][:, 128 * t:128 * (t + 1)], ctx_k[j],
                                 start=(j == 0), stop=(j == 3))
            bst = small.tile([128, 1], F32)
            nc.sync.dma_start(out=bst, in_=bs[128 * t:128 * (t + 1)])
            sct = singles.tile([128, 1], F32, tag=f"sc{t}", name=f"sct{t}")
            nc.vector.tensor_add(sct, pssc, bst)
            sc_sb.append(sct)

        # ---- z = V_dyn @ sc -> [R,1] ----
        psz = pmm.tile([R, 1], F32, tag="pmmtiny")
        for t in range(2):
            nc.tensor.matmul(psz, vdynT[t], sc_sb[t], start=(t == 0), stop=(t == 1))
        z_sb = singles.tile([R, 1], F32)
        nc.scalar.copy(z_sb, psz)

        # ---- scores = U @ z + proto_bias @ sc -> [1, E] ----
        sc_out = singles.tile([1, E], F32)
        for c in range(E // 512):
            pss = pmm.tile([1, 512], F32, tag="pmm512")
            nc.tensor.matmul(pss, z_sb, ut_sb[:, 512 * c:512 * (c + 1)],
                             start=True, stop=False)
            for t in range(2):
                nc.tensor.matmul(pss, sc_sb[t],
                                 pb_sb[t][:, 512 * c:512 * (c + 1)],
                                 start=False, stop=(t == 1))
            nc.scalar.copy(sc_out[:, 512 * c:512 * (c + 1)], pss)
        nc.sync.dma_start(out=scores[:], in_=sc_out)

    nc.compile()
    return nc


# ---------------------------------------------------------------- K2 -------
def build_k2():
    nc = bacc.Bacc()
    X = nc.dram_tensor("X", [N, D], F32, kind="ExternalInput")
    WpT = nc.dram_tensor("WpT", [D, D], F32, kind="ExternalInput")
    bp = nc.dram_tensor("bp", [D], F32, kind="ExternalInput")
    vdyn_s = nc.dram_tensor("vdyn_s", [R, D], F32, kind="ExternalInput")
    UselT = nc.dram_tensor("UselT", [R, KE], F32, kind="ExternalInput")
    bselT = nc.dram_tensor("bselT", [D, KE], F32, kind="ExternalInput")
    idxbc = nc.dram_tensor("idxbc", [128, 4 * KE], I32, kind="ExternalInput")
    edge_w = nc.dram_tensor("edge_w", [N, KE], F32, kind="ExternalOutput")
    edge_idx = nc.dram_tensor("edge_idx", [N, KE], I32, kind="ExternalOutput")

    with tile.TileContext(nc) as tc, ExitStack() as ctx:
        singles = ctx.enter_context(tc.tile_pool(name="singles", bufs=1))
        xpool = ctx.enter_context(tc.tile_pool(name="x", bufs=3))
        xtg_pool = ctx.enter_context(tc.tile_pool(name="xtg", bufs=4))
        xwg_pool = ctx.enter_context(tc.tile_pool(name="xwg", bufs=4))
        ew_pool = ctx.enter_context(tc.tile_pool(name="ew", bufs=3))
        small = ctx.enter_context(tc.tile_pool(name="small", bufs=8))
        ptr = ctx.enter_context(tc.tile_pool(name="ptr", bufs=2, space="PSUM"))
        pshared = ctx.enter_context(tc.tile_pool(name="psh", bufs=2, space="PSUM"))
        plog = ctx.enter_context(tc.tile_pool(name="plog", bufs=3, space="PSUM"))

        id128 = singles.tile([128, 128], F32)
        make_identity(nc, id128)

        wpt = []
        for t in range(2):
            w0 = singles.tile([128, D], F32, tag=f"wpt0_{t}", name=f"wpt0_{t}")
            nc.sync.dma_start(out=w0, in_=WpT[128 * t:128 * (t + 1), :])
            w = singles.tile([128, D], MMDT, tag=f"wpt{t}", name=f"wpt{t}")
            nc.vector.tensor_copy(w, w0)
            wpt.append(w)

        bp_sb = []
        for t in range(2):
            b = singles.tile([128, 1], F32, tag=f"bp{t}", name=f"bpt{t}")
            nc.sync.dma_start(out=b, in_=bp[128 * t:128 * (t + 1)])
            bp_sb.append(b)

        vd0 = singles.tile([R, D], F32)
        nc.sync.dma_start(out=vd0, in_=vdyn_s[:, :])
        vd = singles.tile([R, D], MMDT)
        nc.vector.tensor_copy(vd, vd0)
        us0 = singles.tile([R, KE], F32)
        nc.sync.dma_start(out=us0, in_=UselT[:, :])
        us = singles.tile([R, KE], MMDT)
        nc.vector.tensor_copy(us, us0)

        # replicated top-k indices, pre-broadcast on host to [128, 4*KE]
        idx_sb = singles.tile([128, 4 * KE], I32)
        nc.sync.dma_start(out=idx_sb, in_=idxbc[:, :])

        # protoT [D, KE] scaled by 1/32 (host pre-scales vdyn_s and bselT)
        protoT = []
        for t in range(2):
            bsel = singles.tile([128, KE], F32, tag=f"bsel{t}", name=f"bsel{t}")
            nc.sync.dma_start(out=bsel, in_=bselT[128 * t:128 * (t + 1), :])
            pp = pshared.tile([128, KE], F32, tag="pxw")
            nc.tensor.matmul(pp, vd[:, 128 * t:128 * (t + 1)], us[:, :],
                             start=True, stop=True)
            pt = singles.tile([128, KE], MMDT, tag=f"protoT{t}", name=f"protoT{t}")
            nc.vector.tensor_add(pt, pp, bsel)
            protoT.append(pt)

        for g in range(N // 512):
            # one 512-row X load; partition p holds rows 512g+4p .. +3
            xt = xpool.tile([128, 4, D], F32)
            nc.scalar.dma_start(out=xt, in_=X[512 * g:512 * (g + 1), :])

            # transpose to xtg[t][d, (a,p)]: column 128a+c <-> node 512g+4c+a
            xtg = [xtg_pool.tile([128, 512], MMDT, tag=f"xtg{t}", name=f"xtg{t}")
                   for t in range(2)]
            for a in range(4):
                for t in range(2):
                    pst = ptr.tile([128, 128], F32)
                    nc.tensor.transpose(pst, xt[:, a, 128 * t:128 * (t + 1)], id128)
                    if (a + t) % 2 == 0:
                        nc.vector.tensor_copy(xtg[t][:, 128 * a:128 * (a + 1)], pst)
                    else:
                        nc.scalar.copy(xtg[t][:, 128 * a:128 * (a + 1)], pst)

            # XWp.T for this group: [dout 128t.., 512 cols in (a,p) order]
            xwg = [xwg_pool.tile([128, 512], MMDT, tag=f"xwg{t}", name=f"xwg{t}")
                   for t in range(2)]
            for t in range(2):
                pxw = pshared.tile([128, KE], F32, tag="pxw")
                for j in range(2):
                    nc.tensor.matmul(pxw, wpt[j][:, 128 * t:128 * (t + 1)],
                                     xtg[j], start=(j == 0), stop=(j == 1))
                nc.vector.tensor_scalar_add(xwg[t], pxw, bp_sb[t])

            # logits + softmax; psum rows p <-> node 512g+4p+a
            ew_big = ew_pool.tile([128, 4, KE], F32)
            for a in range(4):
                pl = plog.tile([128, KE], F32)
                for t in range(2):
                    nc.tensor.matmul(pl, xwg[t][:, 128 * a:128 * (a + 1)],
                                     protoT[t], start=(t == 0), stop=(t == 1))
                # |logits| <= ~2 by construction: softmax without max-subtract
                ew = ew_big[:, a, :]
                sumexp = small.tile([128, 1], F32, tag="sumexp")
                nc.scalar.activation(ew, pl, mybir.ActivationFunctionType.Exp,
                                     bias=0.0, scale=1.0, accum_out=sumexp)
                rinv = small.tile([128, 1], F32, tag="rinv")
                nc.vector.reciprocal(rinv, sumexp)
                nc.vector.tensor_scalar_mul(ew, ew, rinv)

            # one store per group per output; 8KB contiguous per partition
            nc.sync.dma_start(out=edge_w[512 * g:512 * (g + 1), :], in_=ew_big)
            nc.scalar.dma_start(out=edge_idx[512 * g:512 * (g + 1), :], in_=idx_sb)

    nc.compile()
    return nc


# ------------------------------------------------------------- host --------
def _ensure_profile_hook():
    """Register the axon NTFF profiling hook if antenv.axon_hooks is absent
    (the boot shim only registers it when the module exists)."""
    import sys
    import types
    try:
        from antenv.axon_hooks import get_axon_ntff_profile_hook  # noqa: F401
        return
    except ImportError:
        pass
    import antenv
    mod = types.ModuleType("antenv.axon_hooks")
    mod._hook = None
    mod.set_axon_ntff_profile_hook = lambda h: setattr(mod, "_hook", h)
    mod.get_axon_ntff_profile_hook = lambda: mod._hook
    sys.modules["antenv.axon_hooks"] = mod
    antenv.axon_hooks = mod
    try:
        from trn_agent_boot.trn_boot import _ntff_profile_via_ctypes
        mod._hook = _ntff_profile_via_ctypes("/opt/axon/libaxon_pjrt.so")
    except Exception:
        mod._hook = None


_CACHE = {}


def _get_k1():
    if "k1" not in _CACHE:
        _CACHE["k1"] = build_k1()
    return _CACHE["k1"]


def _get_k2():
    if "k2" not in _CACHE:
        _CACHE["k2"] = build_k2()
    return _CACHE["k2"]


def kernel(X, U, V, proto_bias, Wc, bc, Ws, bs, Wp, bp, _trace=False):
    X = np.asarray(X, np.float32)
    U = np.asarray(U, np.float32)
    V = np.asarray(V, np.float32)
    proto_bias = np.asarray(proto_bias, np.float32)
    Wc = np.asarray(Wc, np.float32)
    bc = np.asarray(bc, np.float32)
    Ws = np.asarray(Ws, np.float32)
    bs = np.asarray(bs, np.float32)
    Wp = np.asarray(Wp, np.float32)
    bp = np.asarray(bp, np.float32)

    if _trace:
        _ensure_profile_hook()

    WcT = np.ascontiguousarray(Wc.T)
    WsT = np.ascontiguousarray(Ws.T)
    UT = np.ascontiguousarray(U.T)
    pbT = np.ascontiguousarray(proto_bias.T)
    Vbc = np.ascontiguousarray(V + bc.reshape(R, D))

    in_maps1 = []
    for b in range(B):
        in_maps1.append({
            "X": np.ascontiguousarray(X[b]),
            "WcT": WcT, "WsT": WsT, "UT": UT, "pbT": pbT,
            "Vbc": Vbc, "bs": bs,
        })
    res1 = run_bass_kernel_spmd(_get_k1(), in_maps1,
                                core_ids=list(range(NCORES)), trace=_trace)
    t1 = res1.exec_time_ns

    WpT = np.ascontiguousarray(Wp.T)
    in_maps2 = []
    topk = np.empty((B, KE), np.int32)
    for b in range(B):
        s = res1.results[b]["scores"]
        idx = np.argsort(-s, kind="stable")[:KE].astype(np.int32)
        topk[b] = idx
        vdyn_b = res1.results[b]["vdyn"]
        in_maps2.append({
            "X": np.ascontiguousarray(X[b]),
            "WpT": WpT,
            "bp": bp,
            "vdyn_s": np.ascontiguousarray(vdyn_b * SCALE),
            "UselT": np.ascontiguousarray(U[idx].T),
            "bselT": np.ascontiguousarray(proto_bias[idx].T * SCALE),
            "idxbc": np.ascontiguousarray(np.broadcast_to(
                idx[None, None, :], (128, 4, KE)).reshape(128, 4 * KE)),
        })
    res2 = run_bass_kernel_spmd(_get_k2(), in_maps2,
                                core_ids=list(range(NCORES)), trace=_trace)
    t2 = res2.exec_time_ns

    edge_w = np.stack([res2.results[b]["edge_w"] for b in range(B)])
    edge_idx = np.stack([res2.results[b]["edge_idx"] for b in range(B)])

    kernel.last_exec_ns = ((t1 or 0) + (t2 or 0)) or None
    kernel.last_scores = np.stack([res1.results[b]["scores"] for b in range(B)])
    return edge_idx, edge_w, E


kernel.last_exec_ns = None


# revision 19
# speedup vs baseline: 1.1721x; 1.0355x over previous
"""Trainium2 Bass kernel for LowRankSparseHyperedgeGen (moe_routing).

Contract: kernel(**inputs) takes FULL unsharded inputs, returns FULL output
(edge_idx [B,N,kE] i32, edge_w [B,N,kE] f32, E).

Sharding: batch B=8 across the 8 NeuronCores (1 sample/core); all weights
replicated. Two SPMD launches:
  K1: per-sample context (mean/max over nodes), dynamic low-rank factor
      V_dyn, and hyperedge scores [E] (all plain fp32 on device so the
      top-k ranking is bit-stable vs the fp32 reference).
  host: top-512 selection per sample (tiny: argsort of 2048) + row gathers.
  K2: X head-projection (transposed via PE), proto.T reconstruction, logits
      matmul (float32r = full-rate fp32 streaming), softmax, and
      materialization of edge_w / edge_idx.

DMA layout notes: X rows are packed 4-consecutive-rows-per-partition
([128, 4, D] tiles) and outputs are written one 512-row group at a time
([128, 4, KE] -> [512, KE]), so every DMA descriptor covers 4-16KB of
contiguous DRAM. Loads and stores are split between the two HWDGE queues
(sync + scalar) since descriptor generation runs on the issuing engine.
"""

import numpy as np
from contextlib import ExitStack

import concourse.bass as bass
import concourse.bacc as bacc
import concourse.mybir as mybir
import concourse.tile as tile
from concourse.bass_utils import run_bass_kernel_spmd
from concourse.masks import make_identity

F32 = mybir.dt.float32
F32R = mybir.dt.float32r
I32 = mybir.dt.int32

B, N, D, E, R = 8, 4096, 256, 2048, 16
H = 4
KE = 512            # kE = E * 0.25
DH = D // H
SCALE = 1.0 / (np.sqrt(DH) * H)   # 1/32 logit scale
NCORES = 8

# float32r streams fp32 at full PE rate (1 cyc/row when moving dim >= 256);
# inputs must be produced as float32r (compute-op outputs convert). Only K2's
# big matmuls use it; K1 scoring stays plain fp32 for exact top-k ranking.
MMDT = F32R


# ---------------------------------------------------------------- K1 -------
def build_k1():
    nc = bacc.Bacc()
    X = nc.dram_tensor("X", [N, D], F32, kind="ExternalInput")
    WcT = nc.dram_tensor("WcT", [2 * D, R * D], F32, kind="ExternalInput")
    WsT = nc.dram_tensor("WsT", [2 * D, D], F32, kind="ExternalInput")
    UT = nc.dram_tensor("UT", [R, E], F32, kind="ExternalInput")
    pbT = nc.dram_tensor("pbT", [D, E], F32, kind="ExternalInput")
    Vbc = nc.dram_tensor("Vbc", [R, D], F32, kind="ExternalInput")
    bs = nc.dram_tensor("bs", [D], F32, kind="ExternalInput")
    scores = nc.dram_tensor("scores", [E], F32, kind="ExternalOutput")
    vdyn = nc.dram_tensor("vdyn", [R, D], F32, kind="ExternalOutput")

    with tile.TileContext(nc) as tc, ExitStack() as ctx:
        singles = ctx.enter_context(tc.tile_pool(name="singles", bufs=1))
        xpool = ctx.enter_context(tc.tile_pool(name="x", bufs=4))
        small = ctx.enter_context(tc.tile_pool(name="small", bufs=2))
        pred = ctx.enter_context(tc.tile_pool(name="pred", bufs=2, space="PSUM"))
        pmm = ctx.enter_context(tc.tile_pool(name="pmm", bufs=2, space="PSUM"))

        id128 = singles.tile([128, 128], F32)
        make_identity(nc, id128)

        # ---- weights resident in SBUF (sync queue; 8-16KB descriptors) ----
        # X first on BOTH queues (ctx gates everything), then weights:
        # Wc behind X on sync (wdyn consumes it progressively); pb/ws/ut/vbc
        # and every small mid-kernel DMA on scalar, which frees up early.
        # [128, 4, D] tiles: partition p holds rows 512k+4p .. 512k+4p+3.
        acc_sum = singles.tile([128, 4, D], F32)
        acc_max = singles.tile([128, 4, D], F32)
        xts = []
        for k in range(N // 512):
            xt = xpool.tile([128, 4, D], F32)
            eng = nc.sync if k % 2 == 0 else nc.scalar
            eng.dma_start(out=xt, in_=X[512 * k:512 * (k + 1), :])
            xts.append(xt)
        wc_sb = []
        for j in range(4):
            w = singles.tile([128, R * D], F32, tag=f"wc{j}", name=f"wc{j}")
            nc.sync.dma_start(out=w, in_=WcT[128 * j:128 * (j + 1), :])
            wc_sb.append(w)
        pb_sb = []
        for t in range(2):
            w = singles.tile([128, E], F32, tag=f"pb{t}", name=f"pb{t}")
            nc.scalar.dma_start(out=w, in_=pbT[128 * t:128 * (t + 1), :])
            pb_sb.append(w)
        ws_sb = []
        for j in range(4):
            w = singles.tile([128, D], F32, tag=f"ws{j}", name=f"ws{j}")
            nc.scalar.dma_start(out=w, in_=WsT[128 * j:128 * (j + 1), :])
            ws_sb.append(w)
        ut_sb = singles.tile([R, E], F32)
        nc.scalar.dma_start(out=ut_sb, in_=UT[:, :])
        vbc_sb = singles.tile([R, D], F32)
        nc.scalar.dma_start(out=vbc_sb, in_=Vbc[:, :])

        # ---- streaming mean/max over nodes ----
        for k in range(N // 512):
            xt = xts[k]
            if k == 0:
                nc.vector.tensor_copy(acc_sum, xt)
                nc.vector.tensor_copy(acc_max, xt)
            else:
                nc.vector.tensor_add(acc_sum, acc_sum, xt)
                nc.vector.tensor_max(acc_max, acc_max, xt)
        # fold the 4-row sub-axis
        red_sum = singles.tile([128, D], F32)
        red_max = singles.tile([128, D], F32)
        nc.vector.tensor_add(red_sum, acc_sum[:, 0, :], acc_sum[:, 1, :])
        nc.vector.tensor_add(red_sum, red_sum, acc_sum[:, 2, :])
        nc.vector.tensor_add(red_sum, red_sum, acc_sum[:, 3, :])
        nc.vector.tensor_max(red_max, acc_max[:, 0, :], acc_max[:, 1, :])
        nc.vector.tensor_max(red_max, red_max, acc_max[:, 2, :])
        nc.vector.tensor_max(red_max, red_max, acc_max[:, 3, :])

        # ctx vector in partition layout: 4 chunks of [128,1]
        # j=0,1: mean chunks; j=2,3: max chunks
        ctx_k = [singles.tile([128, 1], F32, tag=f"ctx{j}", name=f"ctx{j}")
                 for j in range(4)]
        for t in range(2):
            pst = pred.tile([128, 128], F32, tag="ptr")
            nc.tensor.transpose(pst, red_sum[:, 128 * t:128 * (t + 1)], id128)
            ssum = small.tile([128, 1], F32, tag="ssum")
            nc.vector.tensor_reduce(ssum, pst, axis=mybir.AxisListType.X,
                                    op=mybir.AluOpType.add)
            nc.vector.tensor_scalar_mul(ctx_k[t], ssum, 1.0 / N)
            pst2 = pred.tile([128, 128], F32, tag="ptr")
            nc.tensor.transpose(pst2, red_max[:, 128 * t:128 * (t + 1)], id128)
            nc.vector.tensor_reduce(ctx_k[2 + t], pst2, axis=mybir.AxisListType.X,
                                    op=mybir.AluOpType.max)

        # ---- wdyn = ctx @ Wc.T  -> [1, R*D] (plain fp32 on PE) ----
        wdyn_sb = singles.tile([1, R * D], F32)
        for n in range(8):
            ps = pmm.tile([1, 512], F32, tag="pmm512")
            for j in range(4):
                nc.tensor.matmul(ps, ctx_k[j], wc_sb[j][:, 512 * n:512 * (n + 1)],
                                 start=(j == 0), stop=(j == 3))
            nc.scalar.copy(wdyn_sb[:, 512 * n:512 * (n + 1)], ps)

        # ---- vdyn_nat [R, D] = V + bc.reshape + wdyn.reshape ----
        vnat = singles.tile([R, D], F32)
        for r in range(R):
            nc.scalar.dma_start(out=vnat[r:r + 1, :],
                                in_=wdyn_sb[0:1, D * r:D * (r + 1)])
        nc.vector.tensor_add(vnat, vnat, vbc_sb)
        nc.scalar.dma_start(out=vdyn[:, :], in_=vnat)

        # vdynT chunks [128, R] via PE transpose of vnat
        id16 = id128[:R, :R]
        vdynT = []
        for t in range(2):
            pvt = pred.tile([128, R], F32, tag="ptr")
            nc.tensor.transpose(pvt, vnat[:, 128 * t:128 * (t + 1)], id16)
            vt = singles.tile([128, R], F32, tag=f"vdynT{t}", name=f"vdynT{t}")
            nc.scalar.copy(vt, pvt)
            vdynT.append(vt)

        # ---- sc = ctx @ Ws.T + bs -> [D] in partition chunks [128,1] x2 ----
        sc_sb = []
        for t in range(2):
            pssc = pmm.tile([128, 1], F32, tag="pmmtiny")
            for j in range(4):
                nc.tensor.matmul(pssc, ws_sb[j][:, 128 * t:128 * (t + 1)], ctx_k[j],
                                 start=(j == 0), stop=(j == 3))
            bst = small.tile([128, 1], F32)
            nc.scalar.dma_start(out=bst, in_=bs[128 * t:128 * (t + 1)])
            sct = singles.tile([128, 1], F32, tag=f"sc{t}", name=f"sct{t}")
            nc.vector.tensor_add(sct, pssc, bst)
            sc_sb.append(sct)

        # ---- z = V_dyn @ sc -> [R,1] ----
        psz = pmm.tile([R, 1], F32, tag="pmmtiny")
        for t in range(2):
            nc.tensor.matmul(psz, vdynT[t], sc_sb[t], start=(t == 0), stop=(t == 1))
        z_sb = singles.tile([R, 1], F32)
        nc.scalar.copy(z_sb, psz)

        # ---- scores = U @ z + proto_bias @ sc -> [1, E] ----
        sc_out = singles.tile([1, E], F32)
        for c in range(E // 512):
            pss = pmm.tile([1, 512], F32, tag="pmm512")
            nc.tensor.matmul(pss, z_sb, ut_sb[:, 512 * c:512 * (c + 1)],
                             start=True, stop=False)
            for t in range(2):
                nc.tensor.matmul(pss, sc_sb[t],
                                 pb_sb[t][:, 512 * c:512 * (c + 1)],
                                 start=False, stop=(t == 1))
            nc.scalar.copy(sc_out[:, 512 * c:512 * (c + 1)], pss)
        nc.scalar.dma_start(out=scores[:], in_=sc_out)

    nc.compile()
    return nc


# ---------------------------------------------------------------- K2 -------
def build_k2():
    nc = bacc.Bacc()
    X = nc.dram_tensor("X", [N, D], F32, kind="ExternalInput")
    WpT = nc.dram_tensor("WpT", [D, D], F32, kind="ExternalInput")
    bp = nc.dram_tensor("bp", [D], F32, kind="ExternalInput")
    vdyn_s = nc.dram_tensor("vdyn_s", [R, D], F32, kind="ExternalInput")
    UselT = nc.dram_tensor("UselT", [R, KE], F32, kind="ExternalInput")
    bselT = nc.dram_tensor("bselT", [D, KE], F32, kind="ExternalInput")
    idxbc = nc.dram_tensor("idxbc", [128, 4 * KE], I32, kind="ExternalInput")
    edge_w = nc.dram_tensor("edge_w", [N, KE], F32, kind="ExternalOutput")
    edge_idx = nc.dram_tensor("edge_idx", [N, KE], I32, kind="ExternalOutput")

    with tile.TileContext(nc) as tc, ExitStack() as ctx:
        singles = ctx.enter_context(tc.tile_pool(name="singles", bufs=1))
        xpool = ctx.enter_context(tc.tile_pool(name="x", bufs=3))
        xtg_pool = ctx.enter_context(tc.tile_pool(name="xtg", bufs=4))
        xwg_pool = ctx.enter_context(tc.tile_pool(name="xwg", bufs=4))
        ew_pool = ctx.enter_context(tc.tile_pool(name="ew", bufs=3))
        small = ctx.enter_context(tc.tile_pool(name="small", bufs=8))
        ptr = ctx.enter_context(tc.tile_pool(name="ptr", bufs=2, space="PSUM"))
        pshared = ctx.enter_context(tc.tile_pool(name="psh", bufs=2, space="PSUM"))
        plog = ctx.enter_context(tc.tile_pool(name="plog", bufs=3, space="PSUM"))

        id128 = singles.tile([128, 128], F32)
        make_identity(nc, id128)

        wpt = []
        for t in range(2):
            w0 = singles.tile([128, D], F32, tag=f"wpt0_{t}", name=f"wpt0_{t}")
            nc.sync.dma_start(out=w0, in_=WpT[128 * t:128 * (t + 1), :])
            w = singles.tile([128, D], MMDT, tag=f"wpt{t}", name=f"wpt{t}")
            nc.vector.tensor_copy(w, w0)
            wpt.append(w)

        bp_sb = []
        for t in range(2):
            b = singles.tile([128, 1], F32, tag=f"bp{t}", name=f"bpt{t}")
            nc.sync.dma_start(out=b, in_=bp[128 * t:128 * (t + 1)])
            bp_sb.append(b)

        vd0 = singles.tile([R, D], F32)
        nc.sync.dma_start(out=vd0, in_=vdyn_s[:, :])
        vd = singles.tile([R, D], MMDT)
        nc.vector.tensor_copy(vd, vd0)
        us0 = singles.tile([R, KE], F32)
        nc.sync.dma_start(out=us0, in_=UselT[:, :])
        us = singles.tile([R, KE], MMDT)
        nc.vector.tensor_copy(us, us0)

        # replicated top-k indices, pre-broadcast on host to [128, 4*KE]
        idx_sb = singles.tile([128, 4 * KE], I32)
        nc.sync.dma_start(out=idx_sb, in_=idxbc[:, :])

        # protoT [D, KE] scaled by 1/32 (host pre-scales vdyn_s and bselT)
        protoT = []
        for t in range(2):
            bsel = singles.tile([128, KE], F32, tag=f"bsel{t}", name=f"bsel{t}")
            nc.sync.dma_start(out=bsel, in_=bselT[128 * t:128 * (t + 1), :])
            pp = pshared.tile([128, KE], F32, tag="pxw")
            nc.tensor.matmul(pp, vd[:, 128 * t:128 * (t + 1)], us[:, :],
                             start=True, stop=True)
            pt = singles.tile([128, KE], MMDT, tag=f"protoT{t}", name=f"protoT{t}")
            nc.vector.tensor_add(pt, pp, bsel)
            protoT.append(pt)

        for g in range(N // 512):
            # one 512-row X load; partition p holds rows 512g+4p .. +3
            xt = xpool.tile([128, 4, D], F32)
            nc.scalar.dma_start(out=xt, in_=X[512 * g:512 * (g + 1), :])

            # transpose to xtg[t][d, (a,p)]: column 128a+c <-> node 512g+4c+a
            # 4 transposes share one PSUM bank -> a single [128,512] copy out
            xtg = [xtg_pool.tile([128, 512], MMDT, tag=f"xtg{t}", name=f"xtg{t}")
                   for t in range(2)]
            for t in range(2):
                pst = ptr.tile([128, 4, 128], F32)
                for a in range(4):
                    nc.tensor.transpose(pst[:, a, :],
                                        xt[:, a, 128 * t:128 * (t + 1)], id128)
                if t == 0:
                    nc.vector.tensor_copy(xtg[t], pst)
                else:
                    nc.scalar.copy(xtg[t], pst)

            # XWp.T for this group: [dout 128t.., 512 cols in (a,p) order]
            xwg = [xwg_pool.tile([128, 512], MMDT, tag=f"xwg{t}", name=f"xwg{t}")
                   for t in range(2)]
            for t in range(2):
                pxw = pshared.tile([128, KE], F32, tag="pxw")
                for j in range(2):
                    nc.tensor.matmul(pxw, wpt[j][:, 128 * t:128 * (t + 1)],
                                     xtg[j], start=(j == 0), stop=(j == 1))
                nc.vector.tensor_scalar_add(xwg[t], pxw, bp_sb[t])

            # logits + softmax; psum rows p <-> node 512g+4p+a
            ew_big = ew_pool.tile([128, 4, KE], F32)
            for a in range(4):
                pl = plog.tile([128, KE], F32)
                for t in range(2):
                    nc.tensor.matmul(pl, xwg[t][:, 128 * a:128 * (a + 1)],
                                     protoT[t], start=(t == 0), stop=(t == 1))
                # |logits| <= ~2 by construction: softmax without max-subtract
                ew = ew_big[:, a, :]
                sumexp = small.tile([128, 1], F32, tag="sumexp")
                nc.scalar.activation(ew, pl, mybir.ActivationFunctionType.Exp,
                                     bias=0.0, scale=1.0, accum_out=sumexp)
                rinv = small.tile([128, 1], F32, tag="rinv")
                nc.vector.reciprocal(rinv, sumexp)
                nc.vector.tensor_scalar_mul(ew, ew, rinv)

            # one store per group per output; 8KB contiguous per partition
            nc.sync.dma_start(out=edge_w[512 * g:512 * (g + 1), :], in_=ew_big)
            nc.scalar.dma_start(out=edge_idx[512 * g:512 * (g + 1), :], in_=idx_sb)

    nc.compile()
    return nc


# ------------------------------------------------------------- host --------
def _ensure_profile_hook():
    """Register the axon NTFF profiling hook if antenv.axon_hooks is absent
    (the boot shim only registers it when the module exists)."""
    import sys
    import types
    try:
        from antenv.axon_hooks import get_axon_ntff_profile_hook  # noqa: F401
        return
    except ImportError:
        pass
    import antenv
    mod = types.ModuleType("antenv.axon_hooks")
    mod._hook = None
    mod.set_axon_ntff_profile_hook = lambda h: setattr(mod, "_hook", h)
    mod.get_axon_ntff_profile_hook = lambda: mod._hook
    sys.modules["antenv.axon_hooks"] = mod
    antenv.axon_hooks = mod
    try:
        from trn_agent_boot.trn_boot import _ntff_profile_via_ctypes
        mod._hook = _ntff_profile_via_ctypes("/opt/axon/libaxon_pjrt.so")
    except Exception:
        mod._hook = None


_CACHE = {}


def _get_k1():
    if "k1" not in _CACHE:
        _CACHE["k1"] = build_k1()
    return _CACHE["k1"]


def _get_k2():
    if "k2" not in _CACHE:
        _CACHE["k2"] = build_k2()
    return _CACHE["k2"]


def kernel(X, U, V, proto_bias, Wc, bc, Ws, bs, Wp, bp, _trace=False):
    X = np.asarray(X, np.float32)
    U = np.asarray(U, np.float32)
    V = np.asarray(V, np.float32)
    proto_bias = np.asarray(proto_bias, np.float32)
    Wc = np.asarray(Wc, np.float32)
    bc = np.asarray(bc, np.float32)
    Ws = np.asarray(Ws, np.float32)
    bs = np.asarray(bs, np.float32)
    Wp = np.asarray(Wp, np.float32)
    bp = np.asarray(bp, np.float32)

    if _trace:
        _ensure_profile_hook()

    WcT = np.ascontiguousarray(Wc.T)
    WsT = np.ascontiguousarray(Ws.T)
    UT = np.ascontiguousarray(U.T)
    pbT = np.ascontiguousarray(proto_bias.T)
    Vbc = np.ascontiguousarray(V + bc.reshape(R, D))

    in_maps1 = []
    for b in range(B):
        in_maps1.append({
            "X": np.ascontiguousarray(X[b]),
            "WcT": WcT, "WsT": WsT, "UT": UT, "pbT": pbT,
            "Vbc": Vbc, "bs": bs,
        })
    res1 = run_bass_kernel_spmd(_get_k1(), in_maps1,
                                core_ids=list(range(NCORES)), trace=_trace)
    t1 = res1.exec_time_ns

    WpT = np.ascontiguousarray(Wp.T)
    in_maps2 = []
    topk = np.empty((B, KE), np.int32)
    for b in range(B):
        s = res1.results[b]["scores"]
        idx = np.argsort(-s, kind="stable")[:KE].astype(np.int32)
        topk[b] = idx
        vdyn_b = res1.results[b]["vdyn"]
        in_maps2.append({
            "X": np.ascontiguousarray(X[b]),
            "WpT": WpT,
            "bp": bp,
            "vdyn_s": np.ascontiguousarray(vdyn_b * SCALE),
            "UselT": np.ascontiguousarray(U[idx].T),
            "bselT": np.ascontiguousarray(proto_bias[idx].T * SCALE),
            "idxbc": np.ascontiguousarray(np.broadcast_to(
                idx[None, None, :], (128, 4, KE)).reshape(128, 4 * KE)),
        })
    res2 = run_bass_kernel_spmd(_get_k2(), in_maps2,
                                core_ids=list(range(NCORES)), trace=_trace)
    t2 = res2.exec_time_ns

    edge_w = np.stack([res2.results[b]["edge_w"] for b in range(B)])
    edge_idx = np.stack([res2.results[b]["edge_idx"] for b in range(B)])

    kernel.last_exec_ns = ((t1 or 0) + (t2 or 0)) or None
    kernel.last_scores = np.stack([res1.results[b]["scores"] for b in range(B)])
    return edge_idx, edge_w, E


kernel.last_exec_ns = None
